# revision 41
# baseline (speedup 1.0000x reference)
"""Trainium2 Bass kernel for YOLO-style DetectionLayer decode.

Full input  x: (16, 255, 76, 76) f32  (channel-major: 3 anchors x 85 ch)
Full output  : (16, 17328, 85) f32   (position-major: 3*76*76 rows x 85 ch)

Math per (b, a, gy, gx):
  out[..., 0] = (sigmoid(tx) + gx) * 8
  out[..., 1] = (sigmoid(ty) + gy) * 8
  out[..., 2] = exp(tw) * ANCHOR[a][0]        (stride cancels)
  out[..., 3] = exp(th) * ANCHOR[a][1]
  out[..., 4:] = sigmoid(...)

Sharding: pure data-parallel over batch: 2 batches per core x 8 cores.

Per-core kernel:
  - HW constraint (measured): SBUF DMA writes covering all 128 partitions
    run at ~857ns per 23KB packet; ANY partial-partition write runs at
    ~2x that.  So the 510 input channel rows load as FOUR full-128-row
    f32 tiles at rows [0:128], [128:256], [255:383], [382:510] (2
    duplicate rows) -- minimal traffic at full rate.  t0 is split into
    two column halves on the sync + scalar HWDGE queues so pair 0's
    data lands first; t1/t3 ride gpsimd SWDGE, t2 scalar.  All
    constants pack into two [128, N] tensors (f16 selectors / f32
    tables) so each const DMA is one full-partition batch -- hundreds
    of tiny strided descriptors would clog the HWDGE generator.
  - VectorE casts each tile to fp16 (2e-2 rel-err budget vs fp16's
    ~3e-4): fp16 transposes run the PE at 1 cycle/row (f32 needs 2)
    and a PSUM bank holds 2x the columns.
  - TensorE transposes 46 chunks per (b, a) pair; chunk j takes
    positions {45 p + j} so output partition p holds 45 consecutive
    output rows -> 15.3KB contiguous store runs.  Transpose mode is
    pure routing (square permutation selector; junk rows land in junk
    columns).  Three per-pair cases by where the 85 channels sit:
      * rows 0..84 of one tile: 85-partition operands + 85x85 identity,
        85-col writes at 86-col stride (4B PSUM alignment), 9/bank.
      * rows b..b+84 of one tile (b=42/43): 128-partition operands +
        square perm, 128-col writes at 86-col stride: each write's junk
        tail is overwritten by the next write's real head, 9/bank.
      * split across two tiles: two transposes per chunk into two PSUM
        banks (piece A -> cols 0..42 at 44-stride, piece B -> cols
        0..41 at 42-stride), 15 chunks per bank pair.
  - ScalarE evacuates each bank with fused tanh(v/2) (sigmoid =
    .5+.5*tanh; one ACT table set holds both tanh and exp), plus true
    Exp on the w/h cols straight from PSUM raw values.
  - VectorE: whole-tile affine .5*t+.5 (2x port mode) turns tanh into
    sigmoid; x/y = 8*s + 8*grid (host table); w/h = (2A)*v - A
    (compensating the affine on the exp'd cols).
  - Main stores ride the sync HWDGE queue; the six 16-position tails
    accumulate in one SBUF tile and go out in a single strided store.
"""

import os
import sys

import numpy as np

for _p in ("/opt/trn_rl_repo", "/root/.axon_site/_ro/trn_rl_repo"):
    if os.path.isdir(_p) and _p not in sys.path:
        sys.path.append(_p)

import concourse.bacc as bacc
import concourse.bass as bass
import concourse.mybir as mybir
import concourse.tile as tile
from concourse.bass_utils import run_bass_kernel_spmd

ANCHORS = np.array([[10.0, 13.0], [16.0, 30.0], [33.0, 23.0]], dtype=np.float32)
NB_FULL = 16
N_CORES = 8
NB = NB_FULL // N_CORES  # batches per core
NA = 3
NC = 85  # 5 + 80 channels
NG = 76
NPOS = NG * NG  # 5776
STRIDE = 8.0
NPAIR = NB * NA  # 6

# Position-chunking: output partition p holds rows [45p, 45p+45); chunk j
# gathers positions {45p + j}. 5776 = 128*45 + 16 -> 16-row tail.
RPP = 45  # rows per partition (main part)
MAIN = 128 * RPP  # 5760
TAIL = NPOS - MAIN  # 16

# input tiles: full-128-row loads covering the 510 channel rows
TILE_ROWS = [(0, 128), (128, 256), (255, 383), (382, 510)]
# pair -> how its 85 channels sit in the tiles (see build_program)
PAIR_SRC = [
    ("one", 0, 0),
    ("two", 0, 85, 43, 1, 0),
    ("one", 1, 42),
    ("one", 2, 0),
    ("two", 2, 85, 43, 3, 1),
    ("one", 3, 43),
]

# fp16 PSUM packing strides (byte offsets must stay 4B-aligned)
CS1 = 86  # one-tile paths: 85-col data at 86-col stride, 9 chunks/bank
CSA = 44  # split piece A: 43-col data, 15 chunks/bank
CSB = 42  # split piece B: 42-col data, 15 chunks/bank

F32 = mybir.dt.float32
F16 = mybir.dt.float16
AF = mybir.ActivationFunctionType
OP = mybir.AluOpType


def _groups(cpb):
    return [(g * cpb, min(cpb, RPP - g * cpb)) for g in range(-(-RPP // cpb))]


GYT_CONST = float((MAIN // NG) * STRIDE)  # rows 5760..5775 all have gy=75
assert (MAIN + TAIL - 1) // NG == MAIN // NG


def _gg_table():
    p = np.arange(128)[:, None]
    j = np.arange(RPP)[None, :]
    r = p * RPP + j
    cf = np.zeros((128, 91), dtype=np.float32)
    cf[:, 0:90:2] = (r % NG) * STRIDE
    cf[:, 1:90:2] = (r // NG) * STRIDE
    cf[:TAIL, 90] = ((MAIN + np.arange(TAIL)) % NG) * STRIDE
    return cf


CONSTF_TABLE = _gg_table()

# All five selectors are cyclic shifts cyc(s): P[r, c] = 1 iff
# c == (r - s) mod 128 -- generated on-chip (a DMA'd const would cost
# hundreds of small descriptors that clog the DGE descriptor generator).
# cyc(0)[0:85, 0:85] doubles as the 85x85 identity.  Both split pieces
# route their real channels to HEAD columns: with overwrite packing a
# later write's junk may only land where junk already was.
SEL_SHIFTS = [0, 1, 42, 43, 85]
SEL_B1, SEL_B4, SEL_42, SEL_43, SEL_A = range(5)


def build_program():
    nc = bacc.Bacc(None, target_bir_lowering=False)

    x = nc.dram_tensor("x", (NB, NA * NC, NG, NG), F32, kind="ExternalInput")
    out = nc.dram_tensor("out", (NB, NA * NPOS, NC), F32, kind="ExternalOutput")
    constf = nc.dram_tensor("constf", (128, 91), F32, kind="ExternalInput")

    with tile.TileContext(nc) as tc:
        with (
            tc.tile_pool(name="constp", bufs=1) as constp,
            tc.tile_pool(name="xhp", bufs=1) as xhp,
            tc.tile_pool(name="outp", bufs=3) as outp,
            tc.tile_pool(name="pp", bufs=5, space="PSUM") as pp,
            tc.tile_pool(name="tp", bufs=2, space="PSUM") as tp,
        ):
            # ---- on-chip table generation (GpSimd, emitted before its DMA
            # triggers; no input deps so it runs in the startup window) ----
            I32 = mybir.dt.int32
            chs = constp.tile([128, 5 * 128], F16)
            scratch = constp.tile([128, 128], I32)
            for i, s in enumerate(SEL_SHIFTS):
                # it[r, c] = (s + 128) + c - r; &127 == 0 iff c == (r-s)%128
                nc.gpsimd.iota(
                    scratch[:], [[1, 128]], base=s + 128, channel_multiplier=-1
                )
                nc.vector.tensor_scalar(
                    scratch[:], scratch[:], 127, None, OP.bitwise_and
                )
                nc.vector.tensor_scalar(
                    chs[:, i * 128 : (i + 1) * 128],
                    scratch[:],
                    0,
                    None,
                    OP.is_equal,
                )
            id0s = chs[0:NC, SEL_B1 * 128 : SEL_B1 * 128 + NC]

            def sel(i):
                return chs[:, i * 128 : (i + 1) * 128]

            # grid table (gg + tail gx): one full-128-partition f32 const
            # DMA; its only consumers are the vector fixups (~26us in), so
            # it queues on sync after t0a
            cfs = constp.tile([128, 91], F32)
            ggv = cfs[:, 0:90].rearrange("p (k c) -> p k c", c=2)
            gxts = cfs[0:TAIL, 90:91]

            xf = x.rearrange("b c h w -> (b c) (h w)")

            # four full-128-row tiles loaded as SWDGE cast-DMAs straight
            # to fp16 (f32 HBM read, fp16 SBUF write): no on-chip cast
            # step, half the SBUF write traffic, and the single fast SWDGE
            # queue (~55ns/descriptor) delivers tile k every ~7us so the
            # compute pipeline starts at ~20us.  The grid const rides the
            # otherwise-idle sync queue.
            xhs = [xhp.tile([128, NPOS], F16, name=f"xh{i}") for i in range(4)]
            nc.sync.dma_start(out=cfs[:], in_=constf[:])
            # load order [t0, t3, t1, t2]: pair 5 (t3-only) computes and
            # stores in the early window while the store queue would
            # otherwise idle; the split pairs needing t1/t2 follow
            for i in [0, 3, 1, 2]:
                r0, r1 = TILE_ROWS[i]
                nc.gpsimd.dma_start(out=xhs[i][:], in_=xf[r0:r1, :])

            # all six 16-position tails accumulate here; one store at the end
            tta = constp.tile([TAIL, 512], F32)

            # (tile, 45, 128) chunk views: [:, j, :] = chunk j
            def chunks(t, np_):
                return xhs[t][0:np_, 0:MAIN].rearrange("c (m j) -> c j m", j=RPP)

            for pair in [0, 5, 1, 2, 3, 4]:
                b, a = divmod(pair, NA)
                aw = float(ANCHORS[a, 0])
                ah = float(ANCHORS[a, 1])
                src = PAIR_SRC[pair]
                ot = outp.tile([128, RPP * NC + 1], F32, tag="ot")
                otr = ot[:, 0 : RPP * NC].rearrange("p (k c) -> p k c", c=NC)
                tt = tta[:, pair * NC : (pair + 1) * NC]
                pst = tp.tile([TAIL, 1024], F16, tag="pst")

                if src[0] == "one":
                    t, shift = src[1], src[2]
                    if shift == 0:
                        sq, np_, ow = id0s, NC, NC
                    else:
                        sq = sel(SEL_42 if shift == 42 else SEL_43)
                        np_, ow = 128, 128
                    xm = chunks(t, np_)
                    for k0, nk in _groups(9):
                        ps = pp.tile([128, 1024], F16, tag="ps")
                        for m in range(nk):
                            nc.tensor.transpose(
                                ps[:, CS1 * m : CS1 * m + ow],
                                xm[:, k0 + m, :],
                                sq,
                                tile_position=(0, 0),
                            )
                        psv = ps[:, 0 : nk * CS1].rearrange(
                            "p (k c) -> p k c", c=CS1
                        )
                        nc.scalar.activation(
                            otr[:, k0 : k0 + nk, :],
                            psv[:, :, 0:NC],
                            AF.Tanh,
                            scale=0.5,
                        )
                        nc.scalar.activation(
                            otr[:, k0 : k0 + nk, 2:4], psv[:, :, 2:4], AF.Exp
                        )
                    nc.tensor.transpose(
                        pst[:, 0:ow],
                        xhs[t][0:np_, MAIN:NPOS],
                        sq,
                        tile_position=(0, 0),
                    )
                    nc.scalar.activation(tt, pst[:, 0:NC], AF.Tanh, scale=0.5)
                    nc.scalar.activation(tt[:, 2:4], pst[:, 2:4], AF.Exp)

                else:
                    _, tA, sA, nA, tB, sB = src
                    nB_ = NC - nA
                    selA = sel(SEL_A)
                    selB = sel(SEL_B1 if pair == 1 else SEL_B4)
                    xmA = chunks(tA, 128)
                    xmB = chunks(tB, 128)
                    # piece A (earlier tile) transposes first for ALL
                    # groups: they run while piece B's tile is still
                    # loading, so only the B work trails the load
                    psAs = []
                    for k0, nk in _groups(15):
                        psA = pp.tile([128, 1024], F16, tag="ps")
                        for m in range(nk):
                            nc.tensor.transpose(
                                psA[:, CSA * m : CSA * m + 128],
                                xmA[:, k0 + m, :],
                                selA,
                                tile_position=(0, 0),
                            )
                        psAs.append(psA)
                    for gi, (k0, nk) in enumerate(_groups(15)):
                        psA = psAs[gi]
                        psB = pp.tile([128, 1024], F16, tag="ps")
                        for m in range(nk):
                            nc.tensor.transpose(
                                psB[:, CSB * m : CSB * m + 128],
                                xmB[:, k0 + m, :],
                                selB,
                                tile_position=(0, 0),
                            )
                        pvA = psA[:, 0 : nk * CSA].rearrange(
                            "p (k c) -> p k c", c=CSA
                        )
                        pvB = psB[:, 0 : nk * CSB].rearrange(
                            "p (k c) -> p k c", c=CSB
                        )
                        nc.scalar.activation(
                            otr[:, k0 : k0 + nk, 0:nA],
                            pvA[:, :, 0:nA],
                            AF.Tanh,
                            scale=0.5,
                        )
                        nc.scalar.activation(
                            otr[:, k0 : k0 + nk, nA:NC],
                            pvB[:, :, 0:nB_],
                            AF.Tanh,
                            scale=0.5,
                        )
                        nc.scalar.activation(
                            otr[:, k0 : k0 + nk, 2:4], pvA[:, :, 2:4], AF.Exp
                        )
                    pstB = tp.tile([TAIL, 1024], F16, tag="pst")
                    nc.tensor.transpose(
                        pst[:, 0:128],
                        xhs[tA][0:128, MAIN:NPOS],
                        selA,
                        tile_position=(0, 0),
                    )
                    nc.tensor.transpose(
                        pstB[:, 0:128],
                        xhs[tB][0:128, MAIN:NPOS],
                        selB,
                        tile_position=(0, 0),
                    )
                    nc.scalar.activation(
                        tt[:, 0:nA], pst[:, 0:nA], AF.Tanh, scale=0.5
                    )
                    nc.scalar.activation(
                        tt[:, nA:NC], pstB[:, 0:nB_], AF.Tanh, scale=0.5
                    )
                    nc.scalar.activation(tt[:, 2:4], pst[:, 2:4], AF.Exp)

                # VectorE fixups (main): whole-tile affine at 2x port mode
                # (needs an even element count -> one memset pad column),
                # then per-channel-type corrections.
                nc.vector.memset(ot[:, RPP * NC : RPP * NC + 1], 0.0)
                nc.vector.tensor_scalar(
                    ot[:, 0 : RPP * NC + 1],
                    ot[:, 0 : RPP * NC + 1],
                    0.5,
                    0.5,
                    OP.mult,
                    OP.add,
                )
                xy = otr[:, :, 0:2]
                nc.vector.tensor_scalar(xy, xy, STRIDE, None, OP.mult)
                nc.vector.tensor_tensor(xy, xy, ggv, OP.add)
                wv = otr[:, :, 2:3]
                nc.vector.tensor_scalar(wv, wv, 2.0 * aw, -aw, OP.mult, OP.add)
                hv = otr[:, :, 3:4]
                nc.vector.tensor_scalar(hv, hv, 2.0 * ah, -ah, OP.mult, OP.add)

                # VectorE fixups (tail); odd count (85) -> 84 + last col
                nc.vector.tensor_scalar(
                    tt[:, 0:84], tt[:, 0:84], 0.5, 0.5, OP.mult, OP.add
                )
                nc.vector.tensor_scalar(
                    tt[:, 84:85], tt[:, 84:85], 0.5, 0.5, OP.mult, OP.add
                )
                nc.vector.tensor_scalar(
                    tt[:, 0:1], tt[:, 0:1], STRIDE, gxts[:], OP.mult, OP.add
                )
                nc.vector.tensor_scalar(
                    tt[:, 1:2], tt[:, 1:2], STRIDE, GYT_CONST, OP.mult, OP.add
                )
                nc.vector.tensor_scalar(
                    tt[:, 2:3], tt[:, 2:3], 2.0 * aw, -aw, OP.mult, OP.add
                )
                nc.vector.tensor_scalar(
                    tt[:, 3:4], tt[:, 3:4], 2.0 * ah, -ah, OP.mult, OP.add
                )

                # main store on the sync HWDGE queue: 128 runs of 15.3KB
                obase = a * NPOS
                nc.sync.dma_start(
                    out=out[b, obase : obase + MAIN, :].rearrange(
                        "(p j) c -> p (j c)", p=128
                    ),
                    in_=ot[:, 0 : RPP * NC],
                )

            # one combined tail store: out[b, a*NPOS + 5760 + t, c] with
            # partition t and free (b, a, c) = tta col (b*3+a)*85 + c
            tails = out.rearrange("b (a q) c -> q b a c", a=NA)
            nc.scalar.dma_start(
                out=tails[MAIN:NPOS],
                in_=tta[:, 0 : NPAIR * NC].rearrange(
                    "t (b a c) -> t b a c", b=NB, a=NA
                ),
            )

    nc.compile()
    return nc


_NC_CACHE = None


def _get_program():
    global _NC_CACHE
    if _NC_CACHE is None:
        _NC_CACHE = build_program()
    return _NC_CACHE


def run(x, trace=False, **kwargs):
    """x: full (16, 255, 76, 76) f32. Returns (full_out, BassKernelResults)."""
    x = np.ascontiguousarray(np.asarray(x, dtype=np.float32))
    assert x.shape == (NB_FULL, NA * NC, NG, NG), x.shape
    nc = _get_program()
    in_maps = [
        {
            "x": np.ascontiguousarray(x[c * NB : (c + 1) * NB]),
            "constf": CONSTF_TABLE,
        }
        for c in range(N_CORES)
    ]
    res = run_bass_kernel_spmd(nc, in_maps, list(range(N_CORES)), trace=trace, **kwargs)
    out = np.concatenate([res.results[c]["out"] for c in range(N_CORES)], axis=0)
    return out, res


def kernel(x):
    out, _ = run(x, trace=False)
    return out


# revision 42
# speedup vs baseline: 1.0880x; 1.0880x over previous
"""Trainium2 Bass kernel for YOLO-style DetectionLayer decode.

Full input  x: (16, 255, 76, 76) f32  (channel-major: 3 anchors x 85 ch)
Full output  : (16, 17328, 85) f32   (position-major: 3*76*76 rows x 85 ch)

Math per (b, a, gy, gx):
  out[..., 0] = (sigmoid(tx) + gx) * 8
  out[..., 1] = (sigmoid(ty) + gy) * 8
  out[..., 2] = exp(tw) * ANCHOR[a][0]        (stride cancels)
  out[..., 3] = exp(th) * ANCHOR[a][1]
  out[..., 4:] = sigmoid(...)

Sharding: pure data-parallel over batch: 2 batches per core x 8 cores.

Per-core kernel:
  - HW constraint (measured): SBUF DMA writes covering all 128 partitions
    run at ~857ns per 23KB packet; ANY partial-partition write runs at
    ~2x that.  So the 510 input channel rows load as FOUR full-128-row
    f32 tiles at rows [0:128], [128:256], [255:383], [382:510] (2
    duplicate rows) -- minimal traffic at full rate.  t0 is split into
    two column halves on the sync + scalar HWDGE queues so pair 0's
    data lands first; t1/t3 ride gpsimd SWDGE, t2 scalar.  All
    constants pack into two [128, N] tensors (f16 selectors / f32
    tables) so each const DMA is one full-partition batch -- hundreds
    of tiny strided descriptors would clog the HWDGE generator.
  - VectorE casts each tile to fp16 (2e-2 rel-err budget vs fp16's
    ~3e-4): fp16 transposes run the PE at 1 cycle/row (f32 needs 2)
    and a PSUM bank holds 2x the columns.
  - TensorE transposes 46 chunks per (b, a) pair; chunk j takes
    positions {45 p + j} so output partition p holds 45 consecutive
    output rows -> 15.3KB contiguous store runs.  Transpose mode is
    pure routing (square permutation selector; junk rows land in junk
    columns).  Three per-pair cases by where the 85 channels sit:
      * rows 0..84 of one tile: 85-partition operands + 85x85 identity,
        85-col writes at 86-col stride (4B PSUM alignment), 9/bank.
      * rows b..b+84 of one tile (b=42/43): 128-partition operands +
        square perm, 128-col writes at 86-col stride: each write's junk
        tail is overwritten by the next write's real head, 9/bank.
      * split across two tiles: two transposes per chunk into two PSUM
        banks (piece A -> cols 0..42 at 44-stride, piece B -> cols
        0..41 at 42-stride), 15 chunks per bank pair.
  - ScalarE evacuates each bank with fused tanh(v/2) (sigmoid =
    .5+.5*tanh; one ACT table set holds both tanh and exp), plus true
    Exp on the w/h cols straight from PSUM raw values.
  - VectorE: whole-tile affine .5*t+.5 (2x port mode) turns tanh into
    sigmoid; x/y = 8*s + 8*grid (host table); w/h = (2A)*v - A
    (compensating the affine on the exp'd cols).
  - Main stores ride the sync HWDGE queue; the six 16-position tails
    accumulate in one SBUF tile and go out in a single strided store.
"""

import os
import sys

import numpy as np

for _p in ("/opt/trn_rl_repo", "/root/.axon_site/_ro/trn_rl_repo"):
    if os.path.isdir(_p) and _p not in sys.path:
        sys.path.append(_p)

import concourse.bacc as bacc
import concourse.bass as bass
import concourse.mybir as mybir
import concourse.tile as tile
from concourse.bass_utils import run_bass_kernel_spmd

ANCHORS = np.array([[10.0, 13.0], [16.0, 30.0], [33.0, 23.0]], dtype=np.float32)
NB_FULL = 16
N_CORES = 8
NB = NB_FULL // N_CORES  # batches per core
NA = 3
NC = 85  # 5 + 80 channels
NG = 76
NPOS = NG * NG  # 5776
STRIDE = 8.0
NPAIR = NB * NA  # 6

# Position-chunking: output partition p holds rows [45p, 45p+45); chunk j
# gathers positions {45p + j}. 5776 = 128*45 + 16 -> 16-row tail.
RPP = 45  # rows per partition (main part)
MAIN = 128 * RPP  # 5760
TAIL = NPOS - MAIN  # 16

# input tiles: full-128-row loads covering the 510 channel rows
TILE_ROWS = [(0, 128), (128, 256), (255, 383), (382, 510)]
# pair -> how its 85 channels sit in the tiles (see build_program)
PAIR_SRC = [
    ("one", 0, 0),
    ("two", 0, 85, 43, 1, 0),
    ("one", 1, 42),
    ("one", 2, 0),
    ("two", 2, 85, 43, 3, 1),
    ("one", 3, 43),
]

# fp16 PSUM packing strides (byte offsets must stay 4B-aligned)
CS1 = 86  # one-tile paths: 85-col data at 86-col stride, 9 chunks/bank
CSA = 44  # split piece A: 43-col data, 15 chunks/bank
CSB = 42  # split piece B: 42-col data, 15 chunks/bank

F32 = mybir.dt.float32
F16 = mybir.dt.float16
AF = mybir.ActivationFunctionType
OP = mybir.AluOpType


def _groups(cpb):
    return [(g * cpb, min(cpb, RPP - g * cpb)) for g in range(-(-RPP // cpb))]


GYT_CONST = float((MAIN // NG) * STRIDE)  # rows 5760..5775 all have gy=75
assert (MAIN + TAIL - 1) // NG == MAIN // NG


def _gg_table():
    p = np.arange(128)[:, None]
    j = np.arange(RPP)[None, :]
    r = p * RPP + j
    cf = np.zeros((128, 91), dtype=np.float32)
    cf[:, 0:90:2] = (r % NG) * STRIDE
    cf[:, 1:90:2] = (r // NG) * STRIDE
    cf[:TAIL, 90] = ((MAIN + np.arange(TAIL)) % NG) * STRIDE
    return cf


CONSTF_TABLE = _gg_table()

# All five selectors are cyclic shifts cyc(s): P[r, c] = 1 iff
# c == (r - s) mod 128 -- generated on-chip (a DMA'd const would cost
# hundreds of small descriptors that clog the DGE descriptor generator).
# cyc(0)[0:85, 0:85] doubles as the 85x85 identity.  Both split pieces
# route their real channels to HEAD columns: with overwrite packing a
# later write's junk may only land where junk already was.
SEL_SHIFTS = [0, 1, 42, 43, 85]
SEL_B1, SEL_B4, SEL_42, SEL_43, SEL_A = range(5)


def build_program():
    nc = bacc.Bacc(None, target_bir_lowering=False)

    x = nc.dram_tensor("x", (NB, NA * NC, NG, NG), F32, kind="ExternalInput")
    out = nc.dram_tensor("out", (NB, NA * NPOS, NC), F32, kind="ExternalOutput")
    constf = nc.dram_tensor("constf", (128, 91), F32, kind="ExternalInput")

    with tile.TileContext(nc) as tc:
        with (
            tc.tile_pool(name="constp", bufs=1) as constp,
            tc.tile_pool(name="xhp", bufs=1) as xhp,
            tc.tile_pool(name="outp", bufs=3) as outp,
            tc.tile_pool(name="pp", bufs=5, space="PSUM") as pp,
            tc.tile_pool(name="tp", bufs=2, space="PSUM") as tp,
        ):
            # ---- on-chip table generation (GpSimd, emitted before its DMA
            # triggers; no input deps so it runs in the startup window) ----
            I32 = mybir.dt.int32
            chs = constp.tile([128, 5 * 128], F16)
            scratch = constp.tile([128, 128], I32)
            for i, s in enumerate(SEL_SHIFTS):
                # it[r, c] = (s + 128) + c - r; &127 == 0 iff c == (r-s)%128
                nc.gpsimd.iota(
                    scratch[:], [[1, 128]], base=s + 128, channel_multiplier=-1
                )
                nc.vector.tensor_scalar(
                    scratch[:], scratch[:], 127, None, OP.bitwise_and
                )
                nc.vector.tensor_scalar(
                    chs[:, i * 128 : (i + 1) * 128],
                    scratch[:],
                    0,
                    None,
                    OP.is_equal,
                )
            id0s = chs[0:NC, SEL_B1 * 128 : SEL_B1 * 128 + NC]

            def sel(i):
                return chs[:, i * 128 : (i + 1) * 128]

            # grid table (gg + tail gx): one full-128-partition f32 const
            # DMA; its only consumers are the vector fixups (~26us in), so
            # it queues on sync after t0a
            cfs = constp.tile([128, 91], F32)
            ggv = cfs[:, 0:90].rearrange("p (k c) -> p k c", c=2)
            gxts = cfs[0:TAIL, 90:91]

            xf = x.rearrange("b c h w -> (b c) (h w)")

            # four full-128-row tiles loaded as SWDGE cast-DMAs straight
            # to fp16 (f32 HBM read, fp16 SBUF write): no on-chip cast
            # step, half the SBUF write traffic, and the single fast SWDGE
            # queue (~55ns/descriptor) delivers tile k every ~7us so the
            # compute pipeline starts at ~20us.  The grid const rides the
            # otherwise-idle sync queue.
            xhs = [xhp.tile([128, NPOS], F16, name=f"xh{i}") for i in range(4)]
            nc.sync.dma_start(out=cfs[:], in_=constf[:])
            for i, (r0, r1) in enumerate(TILE_ROWS):
                nc.gpsimd.dma_start(out=xhs[i][:], in_=xf[r0:r1, :])

            # all six 16-position tails accumulate here; one store at the end
            tta = constp.tile([TAIL, 512], F32)

            # (tile, 45, 128) chunk views: [:, j, :] = chunk j
            def chunks(t, np_):
                return xhs[t][0:np_, 0:MAIN].rearrange("c (m j) -> c j m", j=RPP)

            for pair in range(NPAIR):
                b, a = divmod(pair, NA)
                aw = float(ANCHORS[a, 0])
                ah = float(ANCHORS[a, 1])
                src = PAIR_SRC[pair]
                ot = outp.tile([128, RPP * NC + 1], F32, tag="ot")
                otr = ot[:, 0 : RPP * NC].rearrange("p (k c) -> p k c", c=NC)
                tt = tta[:, pair * NC : (pair + 1) * NC]
                pst = tp.tile([TAIL, 1024], F16, tag="pst")

                if src[0] == "one":
                    t, shift = src[1], src[2]
                    if shift == 0:
                        sq, np_, ow = id0s, NC, NC
                    else:
                        sq = sel(SEL_42 if shift == 42 else SEL_43)
                        np_, ow = 128, 128
                    xm = chunks(t, np_)
                    for k0, nk in _groups(9):
                        ps = pp.tile([128, 1024], F16, tag="ps")
                        for m in range(nk):
                            nc.tensor.transpose(
                                ps[:, CS1 * m : CS1 * m + ow],
                                xm[:, k0 + m, :],
                                sq,
                                tile_position=(0, 0),
                            )
                        psv = ps[:, 0 : nk * CS1].rearrange(
                            "p (k c) -> p k c", c=CS1
                        )
                        nc.scalar.activation(
                            otr[:, k0 : k0 + nk, :],
                            psv[:, :, 0:NC],
                            AF.Tanh,
                            scale=0.5,
                        )
                        nc.scalar.activation(
                            otr[:, k0 : k0 + nk, 2:4], psv[:, :, 2:4], AF.Exp
                        )
                    nc.tensor.transpose(
                        pst[:, 0:ow],
                        xhs[t][0:np_, MAIN:NPOS],
                        sq,
                        tile_position=(0, 0),
                    )
                    nc.scalar.activation(tt, pst[:, 0:NC], AF.Tanh, scale=0.5)
                    nc.scalar.activation(tt[:, 2:4], pst[:, 2:4], AF.Exp)

                else:
                    _, tA, sA, nA, tB, sB = src
                    nB_ = NC - nA
                    selA = sel(SEL_A)
                    selB = sel(SEL_B1 if pair == 1 else SEL_B4)
                    xmA = chunks(tA, 128)
                    xmB = chunks(tB, 128)
                    # piece A (earlier tile) transposes first for ALL
                    # groups: they run while piece B's tile is still
                    # loading, so only the B work trails the load
                    psAs = []
                    for k0, nk in _groups(15):
                        psA = pp.tile([128, 1024], F16, tag="ps")
                        for m in range(nk):
                            nc.tensor.transpose(
                                psA[:, CSA * m : CSA * m + 128],
                                xmA[:, k0 + m, :],
                                selA,
                                tile_position=(0, 0),
                            )
                        psAs.append(psA)
                    for gi, (k0, nk) in enumerate(_groups(15)):
                        psA = psAs[gi]
                        psB = pp.tile([128, 1024], F16, tag="ps")
                        for m in range(nk):
                            nc.tensor.transpose(
                                psB[:, CSB * m : CSB * m + 128],
                                xmB[:, k0 + m, :],
                                selB,
                                tile_position=(0, 0),
                            )
                        pvA = psA[:, 0 : nk * CSA].rearrange(
                            "p (k c) -> p k c", c=CSA
                        )
                        pvB = psB[:, 0 : nk * CSB].rearrange(
                            "p (k c) -> p k c", c=CSB
                        )
                        nc.scalar.activation(
                            otr[:, k0 : k0 + nk, 0:nA],
                            pvA[:, :, 0:nA],
                            AF.Tanh,
                            scale=0.5,
                        )
                        nc.scalar.activation(
                            otr[:, k0 : k0 + nk, nA:NC],
                            pvB[:, :, 0:nB_],
                            AF.Tanh,
                            scale=0.5,
                        )
                        nc.scalar.activation(
                            otr[:, k0 : k0 + nk, 2:4], pvA[:, :, 2:4], AF.Exp
                        )
                    pstB = tp.tile([TAIL, 1024], F16, tag="pst")
                    nc.tensor.transpose(
                        pst[:, 0:128],
                        xhs[tA][0:128, MAIN:NPOS],
                        selA,
                        tile_position=(0, 0),
                    )
                    nc.tensor.transpose(
                        pstB[:, 0:128],
                        xhs[tB][0:128, MAIN:NPOS],
                        selB,
                        tile_position=(0, 0),
                    )
                    nc.scalar.activation(
                        tt[:, 0:nA], pst[:, 0:nA], AF.Tanh, scale=0.5
                    )
                    nc.scalar.activation(
                        tt[:, nA:NC], pstB[:, 0:nB_], AF.Tanh, scale=0.5
                    )
                    nc.scalar.activation(tt[:, 2:4], pst[:, 2:4], AF.Exp)

                # VectorE fixups (main): whole-tile affine at 2x port mode
                # (needs an even element count -> one memset pad column),
                # then per-channel-type corrections.
                nc.vector.memset(ot[:, RPP * NC : RPP * NC + 1], 0.0)
                nc.vector.tensor_scalar(
                    ot[:, 0 : RPP * NC + 1],
                    ot[:, 0 : RPP * NC + 1],
                    0.5,
                    0.5,
                    OP.mult,
                    OP.add,
                )
                xy = otr[:, :, 0:2]
                nc.vector.tensor_scalar(xy, xy, STRIDE, None, OP.mult)
                nc.vector.tensor_tensor(xy, xy, ggv, OP.add)
                wv = otr[:, :, 2:3]
                nc.vector.tensor_scalar(wv, wv, 2.0 * aw, -aw, OP.mult, OP.add)
                hv = otr[:, :, 3:4]
                nc.vector.tensor_scalar(hv, hv, 2.0 * ah, -ah, OP.mult, OP.add)

                # VectorE fixups (tail); odd count (85) -> 84 + last col
                nc.vector.tensor_scalar(
                    tt[:, 0:84], tt[:, 0:84], 0.5, 0.5, OP.mult, OP.add
                )
                nc.vector.tensor_scalar(
                    tt[:, 84:85], tt[:, 84:85], 0.5, 0.5, OP.mult, OP.add
                )
                nc.vector.tensor_scalar(
                    tt[:, 0:1], tt[:, 0:1], STRIDE, gxts[:], OP.mult, OP.add
                )
                nc.vector.tensor_scalar(
                    tt[:, 1:2], tt[:, 1:2], STRIDE, GYT_CONST, OP.mult, OP.add
                )
                nc.vector.tensor_scalar(
                    tt[:, 2:3], tt[:, 2:3], 2.0 * aw, -aw, OP.mult, OP.add
                )
                nc.vector.tensor_scalar(
                    tt[:, 3:4], tt[:, 3:4], 2.0 * ah, -ah, OP.mult, OP.add
                )

                # main store on the sync HWDGE queue: 128 runs of 15.3KB
                obase = a * NPOS
                nc.sync.dma_start(
                    out=out[b, obase : obase + MAIN, :].rearrange(
                        "(p j) c -> p (j c)", p=128
                    ),
                    in_=ot[:, 0 : RPP * NC],
                )

            # one combined tail store: out[b, a*NPOS + 5760 + t, c] with
            # partition t and free (b, a, c) = tta col (b*3+a)*85 + c
            tails = out.rearrange("b (a q) c -> q b a c", a=NA)
            nc.scalar.dma_start(
                out=tails[MAIN:NPOS],
                in_=tta[:, 0 : NPAIR * NC].rearrange(
                    "t (b a c) -> t b a c", b=NB, a=NA
                ),
            )

    nc.compile()
    return nc


_NC_CACHE = None


def _get_program():
    global _NC_CACHE
    if _NC_CACHE is None:
        _NC_CACHE = build_program()
    return _NC_CACHE


def run(x, trace=False, **kwargs):
    """x: full (16, 255, 76, 76) f32. Returns (full_out, BassKernelResults)."""
    x = np.ascontiguousarray(np.asarray(x, dtype=np.float32))
    assert x.shape == (NB_FULL, NA * NC, NG, NG), x.shape
    nc = _get_program()
    in_maps = [
        {
            "x": np.ascontiguousarray(x[c * NB : (c + 1) * NB]),
            "constf": CONSTF_TABLE,
        }
        for c in range(N_CORES)
    ]
    res = run_bass_kernel_spmd(nc, in_maps, list(range(N_CORES)), trace=trace, **kwargs)
    out = np.concatenate([res.results[c]["out"] for c in range(N_CORES)], axis=0)
    return out, res


def kernel(x):
    out, _ = run(x, trace=False)
    return out
